# revision 1
# baseline (speedup 1.0000x reference)
"""Distributed Trainium2 kernel for quantized attention (nn_Attention_25812753449411).

Sharding: 16 heads across 8 cores (2 heads/core), batch-of-heads parallel —
no collectives needed. Host prepares dequantized, transposed fp16/bf16
operands per head (with k permuted so a stride-4 sample forms the first 512
columns); device does QK^T (fp16, f32 accum), streaming softmax (sampled
max as exp bias, exp with fused per-row sum on ScalarE), an on-chip xbar
DMA transpose of P into a contiguous [qt, kt, 128] layout, and P^T@V in
ctx^T orientation interleaved through a work queue. Host divides by the
softmax denominator and reassembles the full output.

Math: scores = Qdeq @ Kdeq^T / sqrt(d) with Qdeq = qscale*q + qmin etc.
(exactly the reference's 4-term affine expansion, refactored). softmax is
shift-invariant, so any per-row m_hat <= max works as the exp bias provided
exp(max - m_hat) stays finite; m_hat = max over a stride-4 column sample
covers all but ~0.5% of rows, and the few overflow rows are detected
(inf/nan) and recomputed exactly on the host.
"""

import sys

sys.path.insert(0, "/opt/trn_rl_repo")

import numpy as np
import ml_dtypes

S, B, H, D = 2048, 1, 16, 128
N_CORES = 8
HPC = H // N_CORES  # heads per core
QT = S // 128  # q tiles per head
KT = S // 128  # k tiles per head

_compiled = None


def _build_graph(do_compile=True):
    import concourse.mybir as mybir
    import concourse.tile as tile
    from concourse import bacc

    fp16 = mybir.dt.float16
    bf16 = mybir.dt.bfloat16
    f32 = mybir.dt.float32
    Exp = mybir.ActivationFunctionType.Exp
    Alu = mybir.AluOpType

    nc = bacc.Bacc()

    qdT = nc.declare_dram_parameter("qdT", [HPC, 128, S], fp16, isOutput=False)
    kdT = nc.declare_dram_parameter("kdT", [HPC, 128, S], fp16, isOutput=False)
    vd = nc.declare_dram_parameter("vd", [HPC, 128, KT, 128], bf16, isOutput=False)
    ctxT = nc.declare_dram_parameter("ctxT", [HPC, 128, S], bf16, isOutput=True)
    lsum = nc.declare_dram_parameter("lsum", [HPC, 128, 2 * QT], f32, isOutput=True)

    with tile.TileContext(nc) as tc:
        with (
            tc.tile_pool(name="ins", bufs=1) as ins_pool,
            tc.tile_pool(name="p", bufs=4) as p_pool,
            tc.tile_pool(name="pt", bufs=2) as pt_pool,
            tc.tile_pool(name="stat", bufs=8) as stat_pool,
            tc.tile_pool(name="lout", bufs=1) as lout_pool,
            tc.tile_pool(name="cout", bufs=2) as cout_pool,
            tc.tile_pool(name="ps", bufs=2, space="PSUM") as ps_pool,
            tc.tile_pool(name="subps", bufs=1, space="PSUM") as subps_pool,
            tc.tile_pool(name="cps", bufs=1, space="PSUM") as cps_pool,
        ):
            # per-head PV chunking: only the last head needs the narrow
            # tail (its final chunk is gated by the very last transpose)
            CHUNKS_H = {
                h: [(0, 512), (512, 512), (1024, 512), (1536, 384), (1920, 128)]
                for h in range(HPC)
            }
            NGRP = 4  # kt-groups per chunk (16 kt / 4)

            heads = []
            for h in range(HPC):
                qdT_s = ins_pool.tile([128, S], fp16, tag=f"qdT{h}")
                kdT_s = ins_pool.tile([128, S], fp16, tag=f"kdT{h}")
                vd_s = ins_pool.tile([128, KT, 128], bf16, tag=f"vd{h}")
                if h == 0:
                    # fine-grained HWDGE loads so qt0's sub-matmul starts ASAP
                    nc.sync.dma_start(out=kdT_s[:, 0:512], in_=kdT[h][:, 0:512])
                    nc.sync.dma_start(out=qdT_s[:, 0:128], in_=qdT[h][:, 0:128])
                    nc.sync.dma_start(out=kdT_s[:, 512:], in_=kdT[h][:, 512:])
                    nc.sync.dma_start(out=qdT_s[:, 128:], in_=qdT[h][:, 128:])
                else:
                    nc.gpsimd.dma_start(out=kdT_s[:], in_=kdT[h])
                    nc.gpsimd.dma_start(out=qdT_s[:], in_=qdT[h])
                nc.gpsimd.dma_start(out=vd_s[:], in_=vd[h])
                l_s = lout_pool.tile([128, 2 * QT], f32, tag=f"l{h}")
                pt_s = pt_pool.tile([128, QT, KT, 128], bf16, tag="pt")
                ctx_s = cout_pool.tile([128, S], bf16, tag="ctx")
                heads.append((qdT_s, kdT_s, vd_s, l_s, pt_s, ctx_s))

            ltail = stat_pool.tile([128, 3], f32, tag="ltail")

            # warm the ACT exp table while input DMAs run (reads garbage,
            # result discarded — only the implicit table load matters)
            warm = stat_pool.tile([128, 1], f32, tag="warm")
            nc.scalar.activation(warm[:], warm[:], Exp)

            # PV work queue: (ready_at_global_qtile, h, chunk j, kt-group g).
            # Chunk j needs transposes of qtiles covering its q range; +2
            # slots of slack for SP lag behind ACT.
            pv_queue = []
            for h in range(HPC):
                for j, (q0, w) in enumerate(CHUNKS_H[h]):
                    ready = 16 * h + (q0 + w - 1) // 128 + 2
                    for g in range(NGRP):
                        pv_queue.append((ready, h, j, g))
            pv_queue.sort(key=lambda t: t[0])
            pv_ctx_ps = {}  # (h, j) -> psum tile

            def emit_pv_group(h, j, g):
                _, _, vd_s, _, pt_s, ctx_s = heads[h]
                q0, w = CHUNKS_H[h][j]
                if g == 0:
                    pv_ctx_ps[(h, j)] = cps_pool.tile(
                        [128, w], f32, tag="ctx", name=f"ctxps{h}_{j}",
                        padded_shape=[128, 512],
                    )
                ctx_ps = pv_ctx_ps[(h, j)]
                qt0, qt1 = q0 // 128, (q0 + w) // 128
                for kt in range(4 * g, 4 * g + 4):
                    nc.tensor.matmul(
                        ctx_ps[:],
                        vd_s[:, kt, :],
                        pt_s[:, qt0:qt1, kt, :],
                        start=(kt == 0),
                        stop=(kt == KT - 1),
                    )
                if g == NGRP - 1:
                    nc.vector.tensor_copy(
                        out=ctx_s[:, q0 : q0 + w], in_=ctx_ps[:]
                    )
                    del pv_ctx_ps[(h, j)]
                    if j == len(CHUNKS_H[h]) - 2:
                        # chunks 0..j copied: store the bulk of ctx now
                        nc.gpsimd.dma_start(
                            out=ctxT[h][:, 0 : q0 + w], in_=ctx_s[:, 0 : q0 + w]
                        )
                    elif j == len(CHUNKS_H[h]) - 1:
                        nc.sync.dma_start(
                            out=ctxT[h][:, q0:], in_=ctx_s[:, q0:]
                        )

            order = [(h, qt) for h in range(HPC) for qt in range(QT)]
            # soften the head boundary: swap so h1's first qtiles interleave
            # with h0's last ones
            # (boundary swap disabled — measured neutral-to-negative)
            for gqt, (h, qt) in enumerate(order):
                if True:
                    qdT_s, kdT_s, vd_s, l_s, pt_s, ctx_s = heads[h]
                    # drain one ready PV group per qtile slot
                    if pv_queue and pv_queue[0][0] <= gqt:
                        _, ph, pj, pg = pv_queue.pop(0)
                        emit_pv_group(ph, pj, pg)
                    # --- QK^T: sub (stride-4 sample, permuted-first) + main ---
                    lhs = qdT_s[:, qt * 128 : (qt + 1) * 128]
                    sub_ps = subps_pool.tile([128, 512], f32, tag="sub")
                    nc.tensor.matmul(
                        sub_ps[:], lhs, kdT_s[:, 0:512], start=True, stop=True,
                    )
                    s_ps = ps_pool.tile([128, 1536], f32, tag="S")
                    for j in range(3):
                        nc.tensor.matmul(
                            s_ps[:, j * 512 : (j + 1) * 512],
                            lhs,
                            kdT_s[:, 512 + j * 512 : 512 + (j + 1) * 512],
                            start=True,
                            stop=True,
                        )
                    # --- submax over the sample -> -m_hat bias (margin 0;
                    # rare overflow rows are recomputed on host) ---
                    nm = stat_pool.tile([128, 1], f32, tag="m")
                    nc.vector.reduce_max(
                        nm[:], sub_ps[:], axis=mybir.AxisListType.X, negate=True
                    )
                    # --- exp with fused per-row sums (ScalarE), then the
                    # on-chip transpose P[q,k] -> PT[k_in, kt, q] ---
                    p_t = p_pool.tile([128, S], bf16, tag="p")
                    if h == HPC - 1 and qt == QT - 1:
                        # tail: split exp AND transpose into 512-wide k-pieces
                        # so the trailing PV chunk pipelines with the exp.
                        # accum: piece sums -> l_s[2qt] gets piece0, l_s[2qt+1]
                        # gets pieces 1-3 summed on host? No — host sums the
                        # PAIR; route piece 0 to col 2qt and pieces 1..3 to
                        # distinct scratch columns is not available, so use a
                        # 4-way split of the last l pair via ltail.
                        nc.scalar.activation(
                            p_t[:, 0:512], sub_ps[:], Exp, bias=nm[:], scale=1.0,
                            accum_out=l_s[:, 2 * qt : 2 * qt + 1],
                        )
                        nc.sync.dma_start_transpose(
                            out=pt_s[:, qt, 0:4, :],
                            in_=p_t[:, 0:512],
                        )
                        for piece in range(3):
                            nc.scalar.activation(
                                p_t[:, 512 * (piece + 1) : 512 * (piece + 2)],
                                s_ps[:, 512 * piece : 512 * (piece + 1)], Exp,
                                bias=nm[:], scale=1.0,
                                accum_out=ltail[:, piece : piece + 1],
                            )
                            nc.sync.dma_start_transpose(
                                out=pt_s[:, qt, 4 * (piece + 1) : 4 * (piece + 2), :],
                                in_=p_t[:, 512 * (piece + 1) : 512 * (piece + 2)],
                            )
                        # fold the 3 tail piece-sums into l_s[2qt+1]
                        nc.vector.reduce_sum(
                            l_s[:, 2 * qt + 1 : 2 * qt + 2], ltail[:],
                            axis=mybir.AxisListType.X,
                        )
                    else:
                        nc.scalar.activation(
                            p_t[:, 0:512], sub_ps[:], Exp, bias=nm[:], scale=1.0,
                            accum_out=l_s[:, 2 * qt : 2 * qt + 1],
                        )
                        nc.scalar.activation(
                            p_t[:, 512:2048], s_ps[:], Exp, bias=nm[:], scale=1.0,
                            accum_out=l_s[:, 2 * qt + 1 : 2 * qt + 2],
                        )
                        nc.sync.dma_start_transpose(
                            out=pt_s[:, qt, :, :],
                            in_=p_t[:],
                        )
                    if qt == QT - 1:
                        nc.gpsimd.dma_start(out=lsum[h], in_=l_s[:])

            # drain remaining PV groups (stores are emitted by emit_pv_group)
            while pv_queue:
                _, ph, pj, pg = pv_queue.pop(0)
                emit_pv_group(ph, pj, pg)
    if do_compile:
        nc.compile()
    return nc


def _get_compiled():
    global _compiled
    if _compiled is None:
        _compiled = _build_graph()
    return _compiled


def _prep_core_inputs(c, QdT, KdT, Vd):
    """Slice per-core head shards. QdT/KdT: [H,128,S] f16, Vd: [H,128,KT,128] bf16."""
    hs = slice(c * HPC, (c + 1) * HPC)
    return {
        "qdT": np.ascontiguousarray(QdT[hs]),
        "kdT": np.ascontiguousarray(KdT[hs]),
        "vd": np.ascontiguousarray(Vd[hs]),
    }


def kernel(q, k, v, qmin, qscale, kmin, kscale, vmin, vscale, _trace=False):
    from concourse.bass_utils import run_bass_kernel_spmd

    f32 = np.float32
    q, k, v = np.asarray(q), np.asarray(k), np.asarray(v)
    qmin, qscale = np.asarray(qmin), np.asarray(qscale)
    kmin, kscale = np.asarray(kmin), np.asarray(kscale)
    vmin, vscale = np.asarray(vmin), np.asarray(vscale)
    # [S,B,H,D] -> [H,S,D]
    qh = np.transpose(q.astype(f32), (1, 2, 0, 3))[0]
    kh = np.transpose(k.astype(f32), (1, 2, 0, 3))[0]
    vh = np.transpose(v.astype(f32), (1, 2, 0, 3))[0]

    def col(x):  # [S,B,H,1] -> [H,S,1]
        return np.transpose(x.astype(f32), (1, 2, 0, 3))[0]

    qs, qm = col(qscale), col(qmin)
    ks, km = col(kscale), col(kmin)
    vs, vm = col(vscale), col(vmin)

    inv_sqrt_d = 1.0 / np.sqrt(np.float32(D))
    Qd = (qs * qh + qm) * inv_sqrt_d          # [H,S,D] f32
    Kd = ks * kh + km
    Vd = vs * vh + vm

    QdT = np.ascontiguousarray(Qd.transpose(0, 2, 1)).astype(np.float16)   # [H,128,S]
    # permute k so the stride-4 sample (submax source) is the first 512 columns
    perm = np.concatenate([np.arange(0, S, 4),
                           np.setdiff1d(np.arange(S), np.arange(0, S, 4))])
    KdT = np.ascontiguousarray(Kd.transpose(0, 2, 1)[:, :, perm]).astype(np.float16)
    Vdp = Vd[:, perm, :]
    # Vd [H,S,D] -> [H, k_in(128), kt(KT), d(128)]
    Vd4 = np.ascontiguousarray(
        Vdp.reshape(H, KT, 128, D).transpose(0, 2, 1, 3)
    ).astype(ml_dtypes.bfloat16)

    nc = _get_compiled()
    in_maps = [_prep_core_inputs(c, QdT, KdT, Vd4) for c in range(N_CORES)]
    try:
        res = run_bass_kernel_spmd(nc, in_maps, list(range(N_CORES)), trace=_trace)
    except Exception:
        # transient NRT device errors have been observed once; retry once
        res = run_bass_kernel_spmd(nc, in_maps, list(range(N_CORES)), trace=_trace)
    results = res.results

    out = np.zeros((S, B, H * D), np.float32)
    with np.errstate(invalid="ignore", over="ignore", divide="ignore"):
        for c in range(N_CORES):
            for i in range(HPC):
                h = c * HPC + i
                ctxT_un = results[c]["ctxT"][i].astype(f32)      # [128(d), S(q)]
                ls = results[c]["lsum"][i].astype(f32)           # [128, 2*QT]
                l_full = (ls[:, 0::2] + ls[:, 1::2]).T.reshape(S)  # q = qt*128 + p
                ctx = ctxT_un.T / l_full[:, None]
                # Robustness fallback: a submax gap > 88+MARGIN overflows exp for
                # that row (rare spike). Recompute those rows exactly in f32.
                bad = ~np.isfinite(ctx).all(1) | ~np.isfinite(l_full)
                if bad.any():
                    rows = np.where(bad)[0]
                    Srow = (Qd[h][rows] @ Kd[h].T)               # [n, S] f32
                    Srow -= Srow.max(1, keepdims=True)
                    Prow = np.exp(Srow)
                    ctx[rows] = (Prow @ Vd[h]) / Prow.sum(1, keepdims=True)
                out[:, 0, h * D : (h + 1) * D] = ctx
    if _trace:
        return out, res
    return out

